# revision 6
# baseline (speedup 1.0000x reference)
"""Trainium2 Bass kernel for nn_Block_62156766708387 (moe_routing).

Transformer block: x + attn(LN1(x)), then + top2-MoE(LN2(.)).

Execution plan (8 NeuronCores):
  Launch A  (data-parallel over batch, 1 batch element / core):
      attention in plain fp16 (fp32 PSUM accumulation) -> x1 = xf + y@Wp.
      Empirically (fixed seed) fp16 x1 gives 0/8192 top-2 routing flips
      vs the fp32 reference and final rel_l2 ~1.4e-4.
      Exact bias folds: bk drops (softmax-invariant), bv/bp fold into
      the residual xf = x + bp + bv@Wp on host.
      AV is computed "flipped" (stationary = V augmented with a ones
      column -> row 64 of PSUM is the softmax denominator); the
      denominator reciprocal row is broadcast across partitions with a
      K=1 ones-stationary matmul, and normalization happens in the
      PSUM->SBUF move on DVE.
  Host:     LN2 + gate logits (fp64), top-2 routing, per-expert gather.
  Launch B  (expert-parallel, expert e on core e):
      fp16 FFN y = gelu(tok @ W1 + b1) @ W2 over CAP token slots
      (b2 folded into the host combine).
  Host:     weighted scatter-add + residual.
"""

import numpy as np

import concourse.bass as bass
import concourse.tile as tile
from concourse import bacc, mybir
from concourse import bass_utils
from concourse.bass import ts

F32 = mybir.dt.float32
F16 = mybir.dt.float16

B, T, D = 8, 1024, 1024
H = 4 * D
E = 8
NH, HD = 16, 64
EPS = 1e-5
N_CORES = 8
PT = T // 128    # 8   T tiles
PD = D // 128    # 8   D tiles
PH = H // 128    # 32  H tiles
CAP = 2176       # token slots per expert (max observed count 2158)
CHUNKS = [512, 512, 512, 512, 128]
assert sum(CHUNKS) == CAP

_CACHE = {}


# --------------------------------------------------------------------------
# Launch A: attention block (per-core = one batch element), plain fp16
# --------------------------------------------------------------------------
def _build_attn(reps=1):
    nc = bacc.Bacc("TRN2", target_bir_lowering=False, debug=False,
                   num_devices=N_CORES)
    xf_d = nc.dram_tensor("xf", [T, D], F32, kind="ExternalInput")
    h1t_d = nc.dram_tensor("h1t", [D, T], F16, kind="ExternalInput")
    w_d = {}
    for w in ("wq", "wk", "wv", "wp"):
        w_d[w] = nc.dram_tensor(w, [D, D], F16, kind="ExternalInput")
    bq_d = nc.dram_tensor("bq8", [D], F32, kind="ExternalInput")   # bq/8
    msk_d = nc.dram_tensor("masks", [4, 128, 512], F16, kind="ExternalInput")
    one_d = nc.dram_tensor("ones64", [1, 64], F16, kind="ExternalInput")
    x1_d = nc.dram_tensor("x1", [T, D], F32, kind="ExternalOutput")

    xf_r = xf_d.ap().rearrange("(a p) n -> p a n", p=128)     # [128, 8, 1024]
    h1t_r = h1t_d.ap().rearrange("(a p) t -> p a t", p=128)
    x1_r = x1_d.ap().rearrange("(a p) n -> p a n", p=128)
    w_r = {k: v.ap().rearrange("(k p) n -> p k n", p=128) for k, v in w_d.items()}

    with tile.TileContext(nc) as tc:
        with (
            nc.allow_low_precision(reason="fp16 pipeline validated vs fp32 ref"),
            tc.tile_pool(name="consts", bufs=1) as consts,
            tc.tile_pool(name="wsb", bufs=1) as wsb,
            tc.tile_pool(name="qkv", bufs=1) as qkv,
        ):
            masks = consts.tile([128, 4, 512], F16)
            nc.sync.dma_start(out=masks[:], in_=msk_d.ap().rearrange("m p c -> p m c"))
            ones64 = consts.tile([1, 64], F16)
            nc.sync.dma_start(out=ones64[:], in_=one_d.ap())
            bq_t = consts.tile([128, PD], F32)
            nc.sync.dma_start(out=bq_t[:], in_=bq_d.ap().rearrange("(a p) -> p a", p=128))

            h1 = wsb.tile([128, PD, T], F16)
            for a in range(PD):
                nc.sync.dma_start(out=h1[:, a, :], in_=h1t_r[:, a, :])
            wt = {}
            for w in ("wq", "wk", "wv", "wp"):
                wt[w] = wsb.tile([128, PD, D], F16, name=f"wt_{w}")
                for kk in range(PD):
                    nc.sync.dma_start(out=wt[w][:, kk, :], in_=w_r[w][:, kk, :])

            qT = qkv.tile([128, PD, T], F16)
            kT = qkv.tile([128, PD, T], F16)
            vaug = qkv.tile([128, PT, NH, HD + 1], F16)
            yT = qkv.tile([128, PD, T], F16)
            nc.gpsimd.memset(vaug[:, :, :, HD:HD + 1], 1.0)

            for rep in range(reps):
                # ---------------- Q/K projections ---------------------------
                with (
                    tc.tile_pool(name=f"qtmp{rep}", bufs=3) as qtmp,
                    tc.tile_pool(name=f"psQK{rep}", bufs=3,
                                 space=bass.MemorySpace.PSUM) as psQK,
                ):
                    for wname, dst, scale, use_bias in (
                        ("wq", qT, 0.125, True),
                        ("wk", kT, 1.0, False),
                    ):
                        for j in range(PD):
                            for n in range(T // 512):
                                ps = psQK.tile([128, 512], F32)
                                for kk in range(PD):
                                    nc.tensor.matmul(
                                        ps[:], wt[wname][:, kk, ts(j, 128)],
                                        h1[:, kk, ts(n, 512)],
                                        start=(kk == 0), stop=(kk == PD - 1))
                                if use_bias:
                                    nc.scalar.activation(
                                        dst[:, j, ts(n, 512)], ps[:],
                                        mybir.ActivationFunctionType.Identity,
                                        bias=bq_t[:, j:j + 1], scale=scale)
                                else:
                                    nc.scalar.copy(dst[:, j, ts(n, 512)], ps[:])

                    # --------- V (token-major, fp16, with ones column) ------
                    for i in range(PT):
                        for half in range(2):
                            ps = psQK.tile([128, 512], F32, tag="psv")
                            for kk in range(PD):
                                nc.tensor.matmul(
                                    ps[:], h1[:, kk, ts(i, 128)],
                                    wt["wv"][:, kk, ts(half, 512)],
                                    start=(kk == 0), stop=(kk == PD - 1))
                            nc.scalar.copy(
                                vaug[:, i, 8 * half:8 * half + 8, 0:HD],
                                ps[:].rearrange("p (h c) -> p h c", h=8))

                # ------------- attention: scores -> exp -> AV ---------------
                with (
                    tc.tile_pool(name=f"espool{rep}", bufs=12) as espool,
                    tc.tile_pool(name=f"rcpool{rep}", bufs=4) as rcpool,
                    tc.tile_pool(name=f"psS{rep}", bufs=3,
                                 space=bass.MemorySpace.PSUM) as psS,
                    tc.tile_pool(name=f"psY{rep}", bufs=2,
                                 space=bass.MemorySpace.PSUM) as psY,
                    tc.tile_pool(name=f"psR{rep}", bufs=2,
                                 space=bass.MemorySpace.PSUM) as psR,
                ):
                    for h in range(NH):
                        hp0 = (h % 2) * 64
                        hj = h // 2
                        hsl = slice(hp0, hp0 + 64)
                        for n in range(T // 512):
                            jmax = 4 * (n + 1)
                            blocks = []
                            for j in range(jmax):
                                ps = psS.tile([128, 512], F32)
                                nc.tensor.matmul(ps[:], kT[hsl, hj, ts(j, 128)],
                                                 qT[hsl, hj, ts(n, 512)],
                                                 start=True, stop=True)
                                es = espool.tile([128, 512], F16, tag="es")
                                nc.scalar.activation(
                                    es[:], ps[:],
                                    mybir.ActivationFunctionType.Exp)
                                r = j - 4 * n
                                if r >= 0:
                                    nc.vector.tensor_mul(es[:], es[:],
                                                         masks[:, r, :])
                                blocks.append(es)
                            psy = psY.tile([128, 512], F32)
                            for j in range(jmax):
                                nc.tensor.matmul(psy[0:HD + 1, :],
                                                 vaug[:, j, h, :],
                                                 blocks[j][:],
                                                 start=(j == 0),
                                                 stop=(j == jmax - 1))
                            rc = rcpool.tile([1, 512], F16, tag="rc")
                            nc.vector.reciprocal(rc[:], psy[HD:HD + 1, :])
                            psrc = psR.tile([128, 512], F32)
                            nc.tensor.matmul(psrc[0:HD, :], ones64[:], rc[:],
                                             start=True, stop=True)
                            rcb = rcpool.tile([128, 512], F16, tag="rcb")
                            nc.scalar.copy(rcb[0:HD, :], psrc[0:HD, :])
                            nc.vector.tensor_mul(yT[hsl, hj, ts(n, 512)],
                                                 psy[0:HD, :], rcb[0:HD, :])

                # ---------------- output proj + residual --------------------
                with (
                    tc.tile_pool(name=f"xr{rep}", bufs=4) as xr,
                    tc.tile_pool(name=f"xo{rep}", bufs=4) as xo,
                    tc.tile_pool(name=f"psP{rep}", bufs=3,
                                 space=bass.MemorySpace.PSUM) as psP,
                ):
                    for i in range(PT):
                        for half in range(2):
                            xt = xr.tile([128, 512], F32, tag="xt")
                            nc.sync.dma_start(out=xt[:],
                                              in_=xf_r[:, i, ts(half, 512)])
                            ps = psP.tile([128, 512], F32)
                            for kk in range(PD):
                                nc.tensor.matmul(ps[:], yT[:, kk, ts(i, 128)],
                                                 wt["wp"][:, kk, ts(half, 512)],
                                                 start=(kk == 0),
                                                 stop=(kk == PD - 1))
                            x1t = xo.tile([128, 512], F32, tag="x1t")
                            nc.vector.tensor_add(x1t[:], ps[:], xt[:])
                            nc.sync.dma_start(out=x1_r[:, i, ts(half, 512)],
                                              in_=x1t[:])

    nc.compile()
    return nc


# --------------------------------------------------------------------------
# Launch B: expert FFN (per-core = one expert), fp16, b2 folded on host
# --------------------------------------------------------------------------
def _build_expert(reps=1):
    nc = bacc.Bacc("TRN2", target_bir_lowering=False, debug=False,
                   num_devices=N_CORES)
    tokt_d = nc.dram_tensor("tokt", [D, CAP], F16, kind="ExternalInput")
    w1_d = nc.dram_tensor("w1", [D, H], F16, kind="ExternalInput")
    w2_d = nc.dram_tensor("w2", [H, D], F16, kind="ExternalInput")
    b1_d = nc.dram_tensor("b1", [H], F32, kind="ExternalInput")
    y_d = nc.dram_tensor("y", [CAP, D], F32, kind="ExternalOutput")

    tokt_r = tokt_d.ap().rearrange("(k p) c -> p k c", p=128)
    y_r = y_d.ap().rearrange("(a p) n -> p a n", p=128)

    with tile.TileContext(nc) as tc:
        with (
            tc.tile_pool(name="wpool", bufs=1) as wpool,
            tc.tile_pool(name="consts", bufs=1) as consts,
            tc.tile_pool(name="tokp", bufs=2) as tokp,
            tc.tile_pool(name="midp", bufs=1) as midp,
            tc.tile_pool(name="ysb", bufs=4) as ysbp,
            tc.tile_pool(name="psA", bufs=2, space=bass.MemorySpace.PSUM) as psA,
            tc.tile_pool(name="psB", bufs=2, space=bass.MemorySpace.PSUM) as psB,
        ):
            w1 = wpool.tile([128, PD, H], F16)
            w1r = w1_d.ap().rearrange("(k p) n -> p k n", p=128)
            for kk in range(PD):
                nc.sync.dma_start(out=w1[:, kk, :], in_=w1r[:, kk, :])
            w2 = wpool.tile([128, PH, D], F16)
            w2r = w2_d.ap().rearrange("(k p) n -> p k n", p=128)
            for kk in range(PH):
                nc.sync.dma_start(out=w2[:, kk, :], in_=w2r[:, kk, :])
            b1_t = consts.tile([128, PH], F32)
            nc.sync.dma_start(out=b1_t[:], in_=b1_d.ap().rearrange("(a p) -> p a", p=128))

            for rep in range(reps):
                for ci, cw in enumerate(CHUNKS):
                    c0 = 512 * ci
                    tokc = tokp.tile([128, PD, 512], F16, tag="tok")
                    for kk in range(PD):
                        nc.sync.dma_start(out=tokc[:, kk, :cw],
                                          in_=tokt_r[:, kk, c0:c0 + cw])
                    midc = midp.tile([128, PH, 512], F16, tag="mid")
                    for hj in range(PH):
                        ps = psA.tile([128, 512], F32)
                        for kk in range(PD):
                            nc.tensor.matmul(ps[:, :cw], w1[:, kk, ts(hj, 128)],
                                             tokc[:, kk, :cw],
                                             start=(kk == 0), stop=(kk == PD - 1))
                        nc.scalar.activation(midc[:, hj, :cw], ps[:, :cw],
                                             mybir.ActivationFunctionType.Gelu,
                                             bias=b1_t[:, hj:hj + 1])
                    for ti in range(cw // 128):
                        for nn in range(D // 512):
                            ps2 = psB.tile([128, 512], F32)
                            for hj in range(PH):
                                nc.tensor.matmul(ps2[:], midc[:, hj, ts(ti, 128)],
                                                 w2[:, hj, ts(nn, 512)],
                                                 start=(hj == 0),
                                                 stop=(hj == PH - 1))
                            ysb = ysbp.tile([128, 512], F32, tag="y")
                            nc.scalar.copy(ysb[:], ps2[:])
                            nc.sync.dma_start(
                                out=y_r[:, 4 * ci + ti, ts(nn, 512)], in_=ysb[:])

    nc.compile()
    return nc


# --------------------------------------------------------------------------
# Host-side pieces
# --------------------------------------------------------------------------
def _layernorm64(x, g, b):
    x = x.astype(np.float64)
    mu = x.mean(axis=-1, keepdims=True)
    var = ((x - mu) ** 2).mean(axis=-1, keepdims=True)
    return ((x - mu) / np.sqrt(var + EPS)) * g + b


def _causal_masks():
    m = np.zeros((4, 128, 512), np.float16)
    p = np.arange(128)[:, None]
    c = np.arange(512)[None, :]
    for r in range(4):
        m[r] = (c - p >= r * 128).astype(np.float16)
    return m


def _gelu_exact64(x):
    from math import erf
    v = np.vectorize(erf)
    return 0.5 * x * (1.0 + v(x / np.sqrt(2.0)))


def _get(name, builder):
    if name not in _CACHE:
        _CACHE[name] = builder()
    return _CACHE[name]


def _attn_in_maps(inp):
    x = np.ascontiguousarray(inp["x"], np.float32)
    h1 = _layernorm64(x, inp["ln1_g"].astype(np.float64),
                      inp["ln1_b"].astype(np.float64)).astype(np.float32)
    masks = _causal_masks()
    ones64 = np.ones((1, 64), np.float16)
    wf16 = {nm: np.ascontiguousarray(inp[key], np.float32).astype(np.float16)
            for nm, key in (("wq", "Wq"), ("wk", "Wk"),
                            ("wv", "Wv"), ("wp", "Wp"))}
    bq = inp["bq"].astype(np.float32)
    # exact bias folds: bk is softmax-invariant (dropped); bv/bp fold into
    # the residual:  x1 = (x + bp + bv@Wp) + (P V)@Wp
    fold = (inp["bp"].astype(np.float64)
            + inp["bv"].astype(np.float64) @ inp["Wp"].astype(np.float64))
    in_maps = []
    for b in range(B):
        xf = (x[b].astype(np.float64) + fold).astype(np.float32)
        h1t = np.ascontiguousarray(h1[b].T).astype(np.float16)
        in_maps.append({
            "xf": xf, "h1t": h1t, **wf16,
            "bq8": bq / 8.0,
            "masks": masks, "ones64": ones64,
        })
    return in_maps


def kernel(**inputs):
    inp = {k: np.asarray(v) for k, v in inputs.items()}
    gate_W = inp["gate_W"].astype(np.float64)
    gate_b = inp["gate_b"].astype(np.float64)
    exp_W1 = inp["exp_W1"]
    exp_b1 = inp["exp_b1"]
    exp_W2 = inp["exp_W2"]
    exp_b2 = inp["exp_b2"]

    ncA = _get("attn", _build_attn)
    ncB = _get("expert", _build_expert)

    in_maps_a = _attn_in_maps(inp)
    res_a = bass_utils.run_bass_kernel_spmd(ncA, in_maps_a,
                                            core_ids=list(range(N_CORES)))
    x1 = np.stack([res_a.results[b]["x1"] for b in range(B)])   # [B, T, D] f32

    # ---- host routing ----
    h2_64 = _layernorm64(x1, inp["ln2_g"].astype(np.float64),
                         inp["ln2_b"].astype(np.float64))
    flat = h2_64.reshape(-1, D)                                  # [N, D] f64
    logits = flat @ gate_W + gate_b                              # [N, E] f64
    N = flat.shape[0]
    i1 = np.argmax(logits, axis=1)
    l1 = logits[np.arange(N), i1]
    lm = logits.copy()
    lm[np.arange(N), i1] = -np.inf
    i2 = np.argmax(lm, axis=1)
    l2 = lm[np.arange(N), i2]
    e2 = np.exp(l2 - l1)
    wt1 = (1.0 / (1.0 + e2)).astype(np.float32)
    wt2 = (e2 / (1.0 + e2)).astype(np.float32)

    h2_16 = flat.astype(np.float32).astype(np.float16)
    tok_lists, wgt_lists, ovf = [], [], []
    in_maps_b = []
    for e in range(E):
        sel1 = np.nonzero(i1 == e)[0]
        sel2 = np.nonzero(i2 == e)[0]
        toks = np.concatenate([sel1, sel2])
        wgts = np.concatenate([wt1[sel1], wt2[sel2]])
        if toks.shape[0] > CAP:
            ovf.append((e, toks[CAP:], wgts[CAP:]))
            toks, wgts = toks[:CAP], wgts[:CAP]
        tok_lists.append(toks)
        wgt_lists.append(wgts)
        tokt = np.zeros((D, CAP), np.float16)
        tokt[:, :toks.shape[0]] = h2_16[toks].T
        in_maps_b.append({
            "tokt": tokt,
            "w1": exp_W1[e].astype(np.float16),
            "w2": exp_W2[e].astype(np.float16),
            "b1": exp_b1[e].astype(np.float32),
        })
    res_b = bass_utils.run_bass_kernel_spmd(ncB, in_maps_b,
                                            core_ids=list(range(N_CORES)))

    # ---- combine (b2 folded in here) ----
    moe = np.zeros((N, D), np.float32)
    for e in range(E):
        toks, wgts = tok_lists[e], wgt_lists[e]
        y = res_b.results[e]["y"][:toks.shape[0]] + exp_b2[e].astype(np.float32)
        moe[toks] += wgts[:, None] * y
    for e, toks, wgts in ovf:
        t64 = flat[toks]
        mid = _gelu_exact64(t64 @ exp_W1[e].astype(np.float64)
                            + exp_b1[e].astype(np.float64))
        yv = mid @ exp_W2[e].astype(np.float64) + exp_b2[e].astype(np.float64)
        moe[toks] += wgts[:, None] * yv.astype(np.float32)

    out = x1.reshape(N, D) + moe
    return out.reshape(B, T, D).astype(np.float32)
